# revision 52
# baseline (speedup 1.0000x reference)
"""Distributed causal attention (dense_transformer) for 8 TRN2 NeuronCores.

Sharding: data-parallel over batch (2) x tensor-parallel over heads (16 -> 4
groups of 4). Core c handles batch c//4, heads [4*(c%4), 4*(c%4)+4).

Per core:
  Phase 1 (pipelined per 128-token tile): PE-transpose of x, fused q|k + v
  projections (bf16, wide 512-col matmuls kept back-to-back on the PE so it
  ramps to the 2.4 GHz p-state), rotary (DVE mults + Pool add), PE-transpose
  of rotated q/k into [d, s] layout.
  Phase 2: causal flash-style attention per 1024-query chunk with a
  ones-column folded into V for the softmax denominator (exp on ACT, softmax
  denominator broadcast via PE, normalize on DVE, den copies on Pool).
  Output reduction: instead of a ReduceScatter of 4 MB of out-projection
  partials, each 1024-token chunk's normalized attention output [256, 1024]
  (bf16) is AllToAll'd across the 4-core group (each core keeps a 256-token
  quarter, receiving the other 3 head-groups' slices = full 1024-dim inner
  activations for its tokens), then the core computes the final out-projection
  rows for its own tokens with the full Wout. 4x less collective traffic, and
  the chunk-0 exchange fully overlaps chunk-1 attention.

Each core emits 512 rows ([2 chunks x 256 tokens]) of the final output and
the host concatenates.
"""

import sys

if "/opt/trn_rl_repo" not in sys.path:
    sys.path.insert(0, "/opt/trn_rl_repo")

import math

import numpy as np

import concourse.bass as bass
import concourse.mybir as mybir
import concourse.tile as tile
from concourse import bacc
from concourse.bass import broadcast_tensor_aps
from concourse.bass_utils import run_bass_kernel_spmd
from concourse.masks import make_identity, make_upper_triangular

F32 = mybir.dt.float32
F32R = mybir.dt.float32r
BF16 = mybir.dt.bfloat16
EXP = mybir.ActivationFunctionType.Exp
SIN = mybir.ActivationFunctionType.Sin
MULT = mybir.AluOpType.mult
ADD = mybir.AluOpType.add
BYPASS = mybir.AluOpType.bypass

B, S, D = 2, 2048, 1024
H, DH = 16, 64
HL = 4                      # heads per core
CL = HL * DH                # 256: local inner dim per projection
P = 128
NT = S // P                 # 16 seq tiles
KB = D // P                 # 8 contraction blocks
NCORES = 8
NG = 4                      # cores per replica group
SCALE = DH ** -0.5
IC = 1024                   # attention i-chunk width
NC_CHUNK = S // IC          # 2 chunks
QT = IC // NG               # 256: tokens each core owns per chunk


def _build():
    nc = bacc.Bacc("TRN2", debug=False, num_devices=NCORES)

    xb = nc.dram_tensor("xb", [D, S], BF16, kind="ExternalInput").ap()
    wqk = nc.dram_tensor("wqk", [D, 2 * CL], BF16, kind="ExternalInput").ap()
    wv = nc.dram_tensor("wv", [D, CL], BF16, kind="ExternalInput").ap()
    wo = nc.dram_tensor("wo", [D, D], BF16, kind="ExternalInput").ap()
    rope = nc.dram_tensor("rope", [S, DH], F32, kind="ExternalInput").ap()
    bias = nc.dram_tensor("bias", [1, D], F32R, kind="ExternalInput").ap()
    gsel = nc.dram_tensor("gsel", [P, 2], F32, kind="ExternalInput").ap()
    out_ext = nc.dram_tensor("out", [NC_CHUNK * QT, D], BF16,
                             kind="ExternalOutput").ap()

    with tile.TileContext(nc) as tc:
        _body(nc, tc, xb, wqk, wv, wo, rope, bias, gsel, out_ext)
    nc.compile()
    return nc


def _body(nc, tc, xb, wqk, wv, wo, rope, bias, gsel, out_ext):
    with (
        tc.tile_pool(name="const", bufs=1) as const,
        tc.tile_pool(name="wpool", bufs=1) as wpool,
        tc.tile_pool(name="persist", bufs=1) as persist,
        tc.tile_pool(name="dram", bufs=1, space="DRAM") as dram,
        tc.tile_pool(name="spool", bufs=2, space="PSUM") as spool,
        tc.tile_pool(name="avpool", bufs=1, space="PSUM") as avpool,
        tc.tile_pool(name="opool", bufs=2, space="PSUM") as opool,
    ):
        # ---------------- constants ----------------
        identf = const.tile([P, P], F32)
        make_identity(nc, identf[:])
        identr = const.tile([P, P], F32R)
        nc.vector.tensor_copy(identr[:], identf[:])
        identb = const.tile([P, P], BF16)
        nc.vector.tensor_copy(identb[:], identf[:])
        tri01f = const.tile([P, P], F32)
        make_upper_triangular(nc, tri01f[:], 1.0, diag=True)
        tri01 = const.tile([P, P], BF16)
        nc.vector.tensor_copy(tri01[:], tri01f[:])

        ones4f = const.tile([P, HL], F32)
        nc.vector.memset(ones4f[:], 1.0)
        ones4 = const.tile([P, HL], BF16)
        nc.vector.tensor_copy(ones4[:], ones4f[:])

        onespf = const.tile([1, P], F32)
        nc.vector.memset(onespf[:], 1.0)
        onesp = const.tile([1, P], F32R)
        nc.vector.tensor_copy(onesp[:], onespf[:])

        altsign = const.tile([P, DH], F32)
        nc.vector.memset(altsign[:], 1.0)
        nc.vector.memset(altsign[:].rearrange("p (a r) -> p a r", r=2)[:, :, 0], -1.0)

        # ---------------- weights & rotary tables ----------------
        wqk_sb = wpool.tile([P, KB * 2 * CL], BF16)   # [128k, (kb, 512c)]
        wv_sb = wpool.tile([P, KB * CL], BF16)        # [128k, (kb, 256c)]
        wo_sb = wpool.tile([P, KB * D], BF16)         # [128c, (cb, 1024e)]
        bias_sb = wpool.tile([1, D], F32R)
        gsel_sb = wpool.tile([P, 2], F32)


        cos_sb = wpool.tile([P, NT * DH], F32)
        sgnsin = wpool.tile([P, NT * DH], F32)
        bias_bc = wpool.tile([P, D], F32)

        # ---------------- persistent activations ----------------
        xt = persist.tile([P, KB * S], BF16)   # [128k, (kb, s)] transposed x
        qT = persist.tile([P, 2 * S], BF16)    # [c(2 heads), (ct, s)]
        kT = persist.tile([P, 2 * S], BF16)
        v_sb = persist.tile([P, NT * (CL + HL)], BF16)  # per jt: [4x(64 v | 1)]
        attnT = persist.tile([P, 2 * S], BF16)

        xt3 = xt[:].rearrange("p (kb s) -> p kb s", kb=KB)
        qT3 = qT[:].rearrange("p (c s) -> p c s", c=2)
        kT3 = kT[:].rearrange("p (c s) -> p c s", c=2)
        aT3 = attnT[:].rearrange("p (c s) -> p c s", c=2)
        v3 = v_sb[:].rearrange("p (j h c) -> p j h c", j=NT, h=HL)

        with (
            tc.tile_pool(name="xstage", bufs=2) as xstage,
            tc.tile_pool(name="rstage", bufs=3) as rstage,
            tc.tile_pool(name="qkst", bufs=3) as qkst,
        ):
            # input DMAs in criticality order: leading rope slice + first
            # weight half + leading x chunk first; x arrives pre-transposed
            rope_sb = xstage.tile([P, NT * DH], F32, tag="rope")
            rope4 = rope.rearrange("(t p) d -> p t d", p=P)
            rsb3 = rope_sb[:].rearrange("p (t d) -> p t d", t=NT)
            nc.sync.dma_start(rsb3[:, 0:4, :], rope4[:, 0:4, :])
            wqk4 = wqk.rearrange("(kb p) c -> p kb c", p=P)
            wqk_s3 = wqk_sb[:].rearrange("p (kb c) -> p kb c", kb=KB)
            nc.sync.dma_start(wqk_s3[:, 0:4, :], wqk4[:, 0:4, :])
            xbT = xb.rearrange("(kb p) s -> p kb s", p=P)
            nc.sync.dma_start(xt3[:, :, 0:256], xbT[:, :, 0:256])
            nc.sync.dma_start(wqk_s3[:, 4:8, :], wqk4[:, 4:8, :])
            nc.sync.dma_start(wv_sb[:], wv.rearrange("(kb p) c -> p kb c", p=P))
            nc.sync.dma_start(rsb3[:, 4:NT, :], rope4[:, 4:NT, :])
            for xc in range(1, 8):
                nc.sync.dma_start(
                    xt3[:, :, 256 * xc:256 * (xc + 1)],
                    xbT[:, :, 256 * xc:256 * (xc + 1)],
                )
            nc.sync.dma_start(bias_sb[:], bias[:])
            nc.sync.dma_start(gsel_sb[:], gsel[:, :])
            sin_sb = xstage.tile([P, NT * DH], F32, tag="rsin")
            halfpi = xstage.tile([P, 1], F32, tag="hpi")
            nc.vector.memset(halfpi[:], math.pi / 2)
            s3 = sin_sb[:].rearrange("p (t d) -> p t d", t=NT)
            c3 = cos_sb[:].rearrange("p (t d) -> p t d", t=NT)
            g3 = sgnsin[:].rearrange("p (t d) -> p t d", t=NT)
            a3 = altsign[:].rearrange("p (o d) -> p o d", o=1)
            for t0, t1 in ((0, 4), (4, NT)):
                nc.scalar.activation(c3[:, t0:t1, :], rsb3[:, t0:t1, :], SIN,
                                     bias=halfpi[:])
                nc.scalar.activation(s3[:, t0:t1, :], rsb3[:, t0:t1, :], SIN)
                b0, b1 = broadcast_tensor_aps(s3[:, t0:t1, :], a3)
                nc.vector.tensor_tensor(g3[:, t0:t1, :], b0, b1, op=MULT)

            # pass B: qkv projection + rotary for one token tile
            def pass_b(st):
                sp = spool.tile([P, IC], F32, tag="s")
                for kb in range(KB):
                    nc.tensor.matmul(
                        sp[:, 0:2 * CL],
                        xt3[:, kb, P * st:P * (st + 1)],
                        wqk_sb[:, 2 * CL * kb:2 * CL * (kb + 1)],
                        start=(kb == 0), stop=(kb == KB - 1),
                    )
                for kb in range(KB):
                    nc.tensor.matmul(
                        sp[:, 2 * CL:3 * CL],
                        xt3[:, kb, P * st:P * (st + 1)],
                        wv_sb[:, CL * kb:CL * (kb + 1)],
                        start=(kb == 0), stop=(kb == KB - 1),
                    )
                spv = sp[:, 0:3 * CL]
                sp3 = spv.rearrange("p (h d) -> p h d", h=3 * HL)
                cos_b = cos_sb[:, DH * st:DH * (st + 1)].rearrange(
                    "p (o d) -> p o d", o=1)
                # tcos = qkv * cos
                tcos = rstage.tile([P, 3 * CL], BF16, tag="tc")
                tc3 = tcos[:].rearrange("p (h d) -> p h d", h=3 * HL)
                i0, i1 = broadcast_tensor_aps(sp3, cos_b)
                nc.vector.tensor_tensor(tc3, i0, i1, op=MULT)
                # tsh = rotate_half(qkv) * sgnsin (pair swap via strided AP)
                tsh = rstage.tile([P, 3 * CL], BF16, tag="ts")
                tsh_ap = tsh[:]
                swap_in = bass.AP(
                    tensor=spv.tensor, offset=spv.offset + 1,
                    ap=[list(spv.ap[0]), [DH, 3 * HL], [2, DH // 2], [-1, 2]])
                sg_sl = sgnsin[:, DH * st:DH * (st + 1)]
                sg_in = bass.AP(
                    tensor=sg_sl.tensor, offset=sg_sl.offset,
                    ap=[list(sg_sl.ap[0]), [0, 3 * HL], [2, DH // 2], [1, 2]])
                th_out = bass.AP(
                    tensor=tsh_ap.tensor, offset=tsh_ap.offset,
                    ap=[list(tsh_ap.ap[0]), [DH, 3 * HL], [2, DH // 2], [1, 2]])
                nc.vector.tensor_tensor(th_out, swap_in, sg_in, op=MULT)
                # q|k = tcos + tsh (DVE); v = tcos + tsh (GpSimd, off the
                # critical path) -> v3 with ones column
                qk_s = qkst.tile([P, 2 * CL], BF16, tag="qk")
                nc.vector.tensor_tensor(
                    qk_s[:], tcos[:, 0:2 * CL], tsh[:, 0:2 * CL], op=ADD)
                nc.vector.tensor_tensor(
                    v3[:, st, :, 0:DH],
                    tc3[:, 2 * HL:3 * HL, :], tsh[:].rearrange(
                        "p (h d) -> p h d", h=3 * HL)[:, 2 * HL:3 * HL, :],
                    op=ADD)
                return qk_s

            # pass C: transpose rotated q/k into qT/kT
            def pass_c(st, qk_s):
                tp = opool.tile([P, 512], BF16, tag="o")
                for reg in range(4):
                    nc.tensor.transpose(
                        tp[:, P * reg:P * (reg + 1)],
                        qk_s[:, P * reg:P * (reg + 1)],
                        identb[:],
                    )
                nc.scalar.copy(
                    qT3[:, :, P * st:P * (st + 1)],
                    tp[:, 0:2 * P].rearrange("p (c s) -> p c s", c=2),
                )
                nc.scalar.copy(
                    kT3[:, :, P * st:P * (st + 1)],
                    tp[:, 2 * P:4 * P].rearrange("p (c s) -> p c s", c=2),
                )

            qk_live = {}
            for st in range(NT):
                qk_live[st] = pass_b(st)
                if st >= 1:
                    pass_c(st - 1, qk_live.pop(st - 1))
            pass_c(NT - 1, qk_live.pop(NT - 1))

            # wo load (needed only at projection time) + v ones column
            nc.sync.dma_start(wo_sb[:],
                              wo.rearrange("(cb p) e -> p cb e", p=P))
            for st in range(NT):
                nc.vector.tensor_copy(v3[:, st, :, DH], ones4[:])

            # bias broadcast [1, D] -> [128, D]
            for e2 in range(2):
                bp = opool.tile([P, 512], F32, tag="o")
                for q4 in range(2):
                    nc.tensor.matmul(
                        bp[:, 256 * q4:256 * (q4 + 1)], onesp[:],
                        bias_sb[:, 512 * e2 + 256 * q4:512 * e2 + 256 * (q4 + 1)],
                        start=True, stop=True,
                    )
                nc.vector.tensor_copy(bias_bc[:, 512 * e2:512 * (e2 + 1)],
                                      bp[:, 0:512])

        # ---------------- attention ----------------
        with (
            tc.tile_pool(name="epool", bufs=6) as epool,
            tc.tile_pool(name="rbpool", bufs=3) as rbpool,
            tc.tile_pool(name="aggp", bufs=2) as aggp,
            tc.tile_pool(name="outp", bufs=2) as outp,
        ):
            # A2A payload per (chunk, head-pair): 8 blocks of [128 inner,
            # QT tokens]; block j carries this core's attnT for token
            # quarter j%4 (duplicated to both group halves since the A2A
            # spans both batch groups; receive side masks by group).
            stage_in = [[dram.tile([NCORES * P, QT], BF16, tag=f"si{i}{j}",
                                   name=f"stage_in{i}{j}") for j in range(2)]
                        for i in range(NC_CHUNK)]
            stage_out = [[dram.tile([NCORES * P, QT], BF16, tag=f"so{i}{j}",
                                    name=f"stage_out{i}{j}") for j in range(2)]
                         for i in range(NC_CHUNK)]
            # tiny warmup collective: absorbs the first-op CC stream
            # trigger delay + ramp long before the real exchanges
            wu_in = dram.tile([NCORES, P], BF16, tag="wui", name="wu_in")
            wu_out = dram.tile([NCORES, P], BF16, tag="wuo", name="wu_out")
            nc.gpsimd.collective_compute(
                "AllToAll", BYPASS,
                replica_groups=[[0, 1, 2, 3, 4, 5, 6, 7]],
                ins=[wu_in[:, :].opt()],
                outs=[wu_out[:, :].opt()],
            )

            def attn_chunk(ic, ct_list):
                ibase = IC * ic
                for ct in ct_list:
                    for h in range(2):
                        hl = 2 * ct + h
                        kT_h = kT3[DH * h:DH * (h + 1), ct, :]
                        qT_h = qT3[DH * h:DH * (h + 1), ct, :]
                        av = avpool.tile([DH + 1, IC], F32, tag="av")
                        njt = (ibase + IC) // P
                        for jt in range(njt):
                            jrow = P * jt
                            istart = max(ibase, jrow)
                            w = ibase + IC - istart
                            ioff = istart - ibase
                            chunks = []
                            co = istart
                            while co < ibase + IC:
                                cw = min(512 - (co % 512), ibase + IC - co)
                                chunks.append((co, cw))
                                co += cw
                            sp = spool.tile([P, IC], F32, tag="s")
                            diag = jrow >= ibase
                            for co, cw in chunks:
                                nc.tensor.matmul(
                                    sp[:, co - ibase:co - ibase + cw],
                                    kT_h[:, jrow:jrow + P],
                                    qT_h[:, co:co + cw],
                                    start=True, stop=True,
                                )
                            e = epool.tile([P, IC], BF16, tag="e")
                            nc.scalar.activation(
                                e[:, 0:w], sp[:, ioff:ioff + w], EXP, scale=SCALE)
                            if diag:
                                # zero e where key j > query i on the diag block
                                nc.vector.tensor_tensor(
                                    e[:, 0:P], e[:, 0:P], tri01[:], op=MULT)
                            for co, cw in chunks:
                                jt_last = min(njt - 1, (co + cw - 1) // P)
                                nc.tensor.matmul(
                                    av[:, co - ibase:co - ibase + cw],
                                    v3[:, jt, hl, :],
                                    e[:, co - istart:co - istart + cw],
                                    start=(jt == 0), stop=(jt == jt_last),
                                )
                        # denominator row -> sbuf, broadcast via PE, then a
                        # single divide per 512-chunk on DVE
                        den = rbpool.tile([1, IC], F32R, tag="dn")
                        nc.vector.tensor_copy(den[:], av[DH:DH + 1, :])
                        for q2 in range(IC // 512):
                            rb = opool.tile([P, 512], F32, tag="o")
                            for q4 in range(2):
                                nc.tensor.matmul(
                                    rb[:, 256 * q4:256 * (q4 + 1)], onesp[:],
                                    den[:, 512 * q2 + 256 * q4:
                                        512 * q2 + 256 * (q4 + 1)],
                                    start=True, stop=True,
                                )
                            rbs = rbpool.tile([P, 512], F32, tag="rb")
                            nc.vector.reciprocal_approx_fast(
                                out=rbs[:], in_=rb[:, 0:512])
                            nc.vector.tensor_tensor(
                                aT3[DH * h:DH * (h + 1), ct,
                                    ibase + 512 * q2:ibase + 512 * (q2 + 1)],
                                av[0:DH, 512 * q2:512 * (q2 + 1)],
                                rbs[DH * h:DH * (h + 1), :],
                                op=MULT,
                            )

            def stage_a2a(ic, ct):
                ibase = IC * ic
                si = stage_in[ic][ct]
                for dd in range(2):
                    dst = bass.AP(
                        tensor=si.tensor,
                        offset=si.offset + dd * NG * P * QT,
                        ap=[[QT, P], [P * QT, NG], [1, QT]])
                    nc.sync.dma_start(
                        dst,
                        aT3[:, ct, ibase:ibase + IC].rearrange(
                            "p (q t) -> p q t", q=NG),
                    )
                nc.gpsimd.collective_compute(
                    "AllToAll", BYPASS,
                    replica_groups=[[0, 1, 2, 3, 4, 5, 6, 7]],
                    ins=[si[:, :].opt()],
                    outs=[stage_out[ic][ct][:, :].opt()],
                )

            def proj_chunk(ic):
                # full-inner activations for my QT tokens of chunk ic:
                # rank-half blocks masked by group selector and summed
                half = NG * P
                g0 = aggp.tile([P, KB * QT], BF16, tag="g0")
                g1 = aggp.tile([P, KB * QT], BF16, tag="g1")
                for ct in range(2):
                    g04 = g0[:].rearrange("p (q c t) -> p q c t", q=NG, c=2)
                    g14 = g1[:].rearrange("p (q c t) -> p q c t", q=NG, c=2)
                    nc.sync.dma_start(
                        g04[:, :, ct, :],
                        stage_out[ic][ct][0:half, :].rearrange(
                            "(q p) t -> p q t", p=P))
                    nc.sync.dma_start(
                        g14[:, :, ct, :],
                        stage_out[ic][ct][half:2 * half, :].rearrange(
                            "(q p) t -> p q t", p=P))
                g1m = aggp.tile([P, KB * QT], BF16, tag="g1m")
                nc.vector.tensor_scalar(g1m[:], g1[:], gsel_sb[:, 1:2], None,
                                        op0=MULT)
                aggT = aggp.tile([P, KB * QT], BF16, tag="ag")
                nc.vector.scalar_tensor_tensor(
                    aggT[:], g0[:], gsel_sb[:, 0:1], g1m[:],
                    op0=MULT, op1=ADD)
                agg3 = aggT[:].rearrange("p (cb t) -> p cb t", cb=KB)
                wo3 = wo_sb[:].rearrange("p (cb e) -> p cb e", cb=KB)
                for stq in range(QT // P):
                    o_sb = outp.tile([P, D], BF16, tag="ou")
                    for e2 in range(2):
                        op = opool.tile([P, 512], F32, tag="o")
                        for cb in range(KB):
                            nc.tensor.matmul(
                                op[:],
                                agg3[:, cb, P * stq:P * (stq + 1)],
                                wo3[:, cb, 512 * e2:512 * (e2 + 1)],
                                start=(cb == 0), stop=(cb == KB - 1),
                            )
                        nc.vector.tensor_tensor(
                            o_sb[:, 512 * e2:512 * (e2 + 1)],
                            op[:], bias_bc[:, 512 * e2:512 * (e2 + 1)],
                            op=ADD,
                        )
                    nc.sync.dma_start(
                        out_ext[QT * ic + P * stq:QT * ic + P * (stq + 1), :],
                        o_sb[:],
                    )

            # second tiny warmup right before the exchange phase: resyncs
            # the 8 cores so the real A2As don't absorb accumulated skew
            nc.gpsimd.collective_compute(
                "AllToAll", BYPASS,
                replica_groups=[[0, 1, 2, 3, 4, 5, 6, 7]],
                ins=[wu_in[:, :].opt()],
                outs=[wu_out[:, :].opt()],
            )
            # short unit first so the CC stream starts early; each later
            # exchange gets a >=30us compute window to hide under
            attn_chunk(0, [0])
            stage_a2a(0, 0)
            attn_chunk(1, [0])
            stage_a2a(1, 0)
            attn_chunk(1, [1])
            stage_a2a(1, 1)
            attn_chunk(0, [1])
            stage_a2a(0, 1)
            proj_chunk(1)
            proj_chunk(0)


_NC = None


def _get_nc():
    global _NC
    if _NC is None:
        _NC = _build()
    return _NC


def _in_maps(x, rotary_pos_emb, Wqkv, Wout, bout):
    import ml_dtypes
    bf16 = ml_dtypes.bfloat16
    # pre-transposed per batch: [D, S] bf16
    xT = [np.ascontiguousarray(np.asarray(x[b], np.float32).astype(bf16).T)
          for b in range(B)]
    Wqkv = np.asarray(Wqkv, dtype=np.float32).astype(bf16)
    Wout = np.ascontiguousarray(np.asarray(Wout, np.float32).astype(bf16))
    rope = np.ascontiguousarray(rotary_pos_emb, dtype=np.float32)
    bout = np.ascontiguousarray(bout, dtype=np.float32).reshape(1, D)
    maps = []
    for c in range(NCORES):
        b, hg = c // 4, c % 4
        base = hg * CL
        wqk = np.concatenate(
            [Wqkv[:, base:base + CL], Wqkv[:, D + base:D + base + CL]], axis=1)
        gsel = np.zeros((P, 2), dtype=np.float32)
        gsel[:, b] = 1.0
        maps.append({
            "xb": xT[b],
            "wqk": np.ascontiguousarray(wqk),
            "wv": np.ascontiguousarray(Wqkv[:, 2 * D + base:2 * D + base + CL]),
            "wo": Wout,
            "rope": rope,
            "bias": bout,
            "gsel": gsel,
        })
    return maps


def _run(x, mask, rotary_pos_emb, Wqkv, Wout, bout, trace=False):
    nc = _get_nc()
    maps = _in_maps(x, rotary_pos_emb, Wqkv, Wout, bout)
    res = run_bass_kernel_spmd(nc, maps, core_ids=list(range(NCORES)),
                               trace=trace)
    out = np.empty((B, S, D), dtype=np.float32)
    # core c = 4b + r owns tokens [256r, 256r+256) of each 1024-token chunk
    for c in range(NCORES):
        b, r = c // 4, c % 4
        o = res.results[c]["out"]
        for t in range(NC_CHUNK):
            out[b, IC * t + QT * r:IC * t + QT * (r + 1), :] = \
                o[QT * t:QT * (t + 1), :]
    return out, res


def kernel(x, mask, rotary_pos_emb, Wqkv, Wout, bout):
    out, _ = _run(x, mask, rotary_pos_emb, Wqkv, Wout, bout, trace=False)
    return out
